# revision 1
# baseline (speedup 1.0000x reference)
"""Trainium2 Bass kernel for nn_DeltaModel (DeltaNet-style memory scan).

Math reduction (exact):
  - h = LN(e + FF(e)) depends only on the token id (V=64 vocab) -> 64-row
    table h_table; the (B, L, H) activation tensor is never materialized.
  - Only ctx = M_final @ q is needed.  With M_t = M_{t-1}(I - b_t k_t k_t^T)
    + k_t k_t^T, propagating u backwards (u <- u - b_t k_t (k_t . u), from
    u = q) gives ctx = sum_t k_t (k_t . u_t): O(H) per step instead of the
    O(H^2) matrix scan.  ctx is accumulated in vocab space as
    z = sum_t d_t e_{tok_t} and post-multiplied once:
      out = z @ (h_table @ read_w @ out_w) + (read_b @ out_w + out_b)
  - Sharding: pure data parallel over B (256 -> 32 rows per core).

Device kernel per core: per step two DVE scalar_tensor_tensor ops on the
(32, 128) state S = [u | z]:  dot (accum_out -> -d_t), then fused update
S += -d_t * [beta*k | -onehot].  The per-token
[k | beta*k | -onehot] rows are gathered ON DEVICE from a 65-row DRAM table
with dma_gather (row 64 is zeros for the tail pad), so the host ships only
int16 indices (2 MB/core) instead of a 100 MB expanded stream.

dma_gather writes index j to partition j%128; with j = s*32 + b, step s lives
in partition band 32*(s%4)..+32.  DVE requires equal base partitions for both
SBUF inputs, so the scan state rotates bands each step: inputs (chunk slice,
S_cur, dneg) share band g(s) = 32*(s%4) and the updated state is written to
band g(s+1) -- outputs may have a different base (verified on HW).

Readout-constant DMAs (ftab/gbias/ident) are issued after the scan loop so
the first idx DMA + gather are not queued behind them on the HWDGE FIFO.
"""

import os

import numpy as np

import concourse.bass as bass
from concourse import bacc
import concourse.tile as tile
from concourse import mybir
from concourse.bass_utils import run_bass_kernel_spmd

B, L, H, V = 256, 4096, 64, 64
N_CORES = 8
B_LOC = B // N_CORES
LN_EPS = 1e-5

NSTEP = L - 1
CK = 64                  # steps per chunk
NSTEP_PAD = ((NSTEP + CK - 1) // CK) * CK
NCHUNK = NSTEP_PAD // CK
ROW = 192                # [k | beta*k | -onehot] f32
NIDX = CK * B_LOC        # gather indices per chunk (2048)

FP = mybir.dt.float32
I16 = mybir.dt.int16


def _build_program():
    nc = bacc.Bacc(None, target_bir_lowering=False, debug=False)

    rowtab_d = nc.dram_tensor("rowtab", [V + 1, ROW], FP, kind="ExternalInput").ap()
    idx_d = nc.dram_tensor(
        "idx", [NCHUNK, 128, NIDX // 16], I16, kind="ExternalInput"
    ).ap()
    qz_d = nc.dram_tensor("qz", [B_LOC, 2 * H], FP, kind="ExternalInput").ap()
    ftab_d = nc.dram_tensor("ftab", [V, H], FP, kind="ExternalInput").ap()
    gbias_d = nc.dram_tensor("gbias", [H, 1], FP, kind="ExternalInput").ap()
    ident_d = nc.dram_tensor("ident", [B_LOC, B_LOC], FP, kind="ExternalInput").ap()
    out_d = nc.dram_tensor("out_t", [H, B_LOC], FP, kind="ExternalOutput").ap()

    with tile.TileContext(nc) as tc:
        with (
            tc.tile_pool(name="chunks", bufs=3) as chunk_pool,
            tc.tile_pool(name="idxp", bufs=3) as idx_pool,
            tc.tile_pool(name="state", bufs=2) as state_pool,
            tc.tile_pool(name="small", bufs=2) as small_pool,
            tc.tile_pool(name="consts", bufs=1) as const_pool,
            tc.tile_pool(name="psum", bufs=2, space=bass.MemorySpace.PSUM) as psum_pool,
        ):
            trash = const_pool.tile([128, H], FP, tag="trash")

            s_cur = None
            for c in range(NCHUNK):
                idx = idx_pool.tile([128, NIDX // 16], I16, tag="idx")
                nc.sync.dma_start(idx[:], idx_d[c])
                chunk = chunk_pool.tile([128, NIDX // 128, ROW], FP, tag="chunk")
                # finer splits on chunk 0: first band ready sooner
                NSPLIT = 16 if c == 0 else 4
                NI = NIDX // NSPLIT
                for q in range(NSPLIT):
                    nc.gpsimd.dma_gather(
                        chunk[:, q * (NI // 128) : (q + 1) * (NI // 128), :],
                        rowtab_d[:],
                        idx[:, q * (NI // 16) : (q + 1) * (NI // 16)],
                        num_idxs=NI, num_idxs_reg=NI, elem_size=ROW,
                    )
                if s_cur is None:
                    s_cur = state_pool.tile([128, 2 * H], FP, tag="S")
                    nc.sync.dma_start(s_cur[0:B_LOC, :], qz_d[:])
                for s in range(CK):
                    g = B_LOC * (s % 4)
                    gn = B_LOC * ((s + 1) % 4)
                    r = s // 4
                    dneg = small_pool.tile([128, 1], FP, tag="dneg")
                    nc.vector.scalar_tensor_tensor(
                        out=trash[g : g + B_LOC, :],
                        in0=chunk[g : g + B_LOC, r, 0:H],
                        scalar=-1.0,
                        in1=s_cur[g : g + B_LOC, 0:H],
                        op0=mybir.AluOpType.mult,
                        op1=mybir.AluOpType.mult,
                        accum_out=dneg[g : g + B_LOC, :],
                    )
                    s_new = state_pool.tile([128, 2 * H], FP, tag="S")
                    nc.vector.scalar_tensor_tensor(
                        out=s_new[gn : gn + B_LOC, :],
                        in0=chunk[g : g + B_LOC, r, H:ROW],
                        scalar=dneg[g : g + B_LOC, :],
                        in1=s_cur[g : g + B_LOC, :],
                        op0=mybir.AluOpType.mult,
                        op1=mybir.AluOpType.add,
                    )
                    s_cur = s_new

            ftab = const_pool.tile([V, H], FP, tag="ftab")
            nc.sync.dma_start(ftab[:], ftab_d[:])
            gbias = const_pool.tile([H, 1], FP, tag="gbias")
            nc.sync.dma_start(gbias[:], gbias_d[:])
            ident = const_pool.tile([B_LOC, B_LOC], FP, tag="ident")
            nc.sync.dma_start(ident[:], ident_d[:])

            # final state is in band 0 (NSTEP_PAD % 4 == 0)
            zt_ps = psum_pool.tile([2 * H, B_LOC], FP, tag="zt")
            nc.tensor.transpose(zt_ps[:], s_cur[0:B_LOC, :], ident[:])
            zt = const_pool.tile([H, B_LOC], FP, tag="zts")
            nc.vector.tensor_copy(zt[:], zt_ps[H : 2 * H, :])
            o_ps = psum_pool.tile([H, B_LOC], FP, tag="ops")
            nc.tensor.matmul(o_ps[:], ftab[:], zt[:], start=True, stop=True)
            o_sb = const_pool.tile([H, B_LOC], FP, tag="osb")
            nc.vector.tensor_scalar_add(o_sb[:], o_ps[:], gbias[:])
            nc.sync.dma_start(out_d[:], o_sb[:])

    nc.compile()
    return nc


_PROGRAM_CACHE = {}


def _get_program():
    if "nc" not in _PROGRAM_CACHE:
        _PROGRAM_CACHE["nc"] = _build_program()
    return _PROGRAM_CACHE["nc"]


def _host_tables(embed_W, ff_w1, ff_b1, ff_w2, ff_b2, ln_w, ln_b,
                 read_w, read_b, out_w, out_b):
    """Token-level tables: input-independent (V=64 rows through the MLP+LN)."""
    e = embed_W.astype(np.float64)
    ff = np.maximum(e @ ff_w1 + ff_b1, 0.0) @ ff_w2 + ff_b2
    x = e + ff
    mu = x.mean(-1, keepdims=True)
    var = ((x - mu) ** 2).mean(-1, keepdims=True)
    h_table = (x - mu) / np.sqrt(var + LN_EPS) * ln_w + ln_b
    beta = 1.0 / ((h_table ** 2).sum(-1) + 1e-6)
    F = h_table @ read_w.astype(np.float64) @ out_w.astype(np.float64)
    g = read_b.astype(np.float64) @ out_w.astype(np.float64) + out_b
    return h_table, beta, F, g


def kernel(seq, embed_W, ff_w1, ff_b1, ff_w2, ff_b2, ln_w, ln_b,
           read_w, read_b, out_w, out_b):
    seq = np.asarray(seq)
    h_table, beta, F, g = _host_tables(
        np.asarray(embed_W), np.asarray(ff_w1), np.asarray(ff_b1),
        np.asarray(ff_w2), np.asarray(ff_b2), np.asarray(ln_w),
        np.asarray(ln_b), np.asarray(read_w), np.asarray(read_b),
        np.asarray(out_w), np.asarray(out_b))

    rowtab = np.zeros((V + 1, ROW), np.float32)
    rowtab[:V, 0:H] = h_table
    rowtab[:V, H : 2 * H] = beta[:, None] * h_table
    rowtab[:V, 2 * H : 2 * H + V] = -np.eye(V)

    ftab_in = np.ascontiguousarray(F.astype(np.float32))
    gbias_in = np.ascontiguousarray(g.astype(np.float32)[:, None])
    ident_in = np.eye(B_LOC, dtype=np.float32)

    nc = _get_program()
    in_maps = []
    for c in range(N_CORES):
        tok = seq[c * B_LOC : (c + 1) * B_LOC]            # (32, L)
        tok_rev = tok[:, NSTEP - 1 :: -1]                 # t = L-2 .. 0
        vals = np.full((NSTEP_PAD, B_LOC), V, np.int16)   # pad -> zero row
        vals[:NSTEP] = tok_rev.T                          # j = s*32 + b order
        vals = vals.reshape(NCHUNK, NIDX)                 # per-chunk j-major
        # wrap: index j at (partition j%16, col j//16), replicated x8
        wrapped = vals.reshape(NCHUNK, NIDX // 16, 16).transpose(0, 2, 1)
        idx_in = np.tile(wrapped, (1, 8, 1))              # (NCHUNK, 128, NIDX//16)
        q = h_table[tok[:, L - 1]].astype(np.float32)
        qz = np.concatenate([q, np.zeros((B_LOC, H), np.float32)], axis=1)
        in_maps.append(
            {
                "rowtab": rowtab,
                "idx": np.ascontiguousarray(idx_in),
                "qz": qz,
                "ftab": ftab_in,
                "gbias": gbias_in,
                "ident": ident_in,
            }
        )

    res = run_bass_kernel_spmd(
        nc, in_maps, list(range(N_CORES)),
        trace=bool(int(os.environ.get("KERNEL_TRACE", "0"))),
    )
    if res.exec_time_ns is not None:
        print(f"HW exec time: {res.exec_time_ns} ns")

    out = np.concatenate(
        [res.results[c]["out_t"].T for c in range(N_CORES)], axis=0
    )
    return out.astype(np.float32)

